# revision 12
# baseline (speedup 1.0000x reference)
"""GAT regressor (2x GATConv + mean-pool + MLP) on 8 Trainium2 cores. v2.

Single-launch fused design (dst-sharded, aggregate-then-transform), improved
over v1:
- Message tables T1/T2 are BF16 (records [x(16),a_s1(4)] and
  [h2(32),a_s2(1),pad(3)]): halves the AllGather payloads and the per-edge
  gather bytes; DVE ops read the bf16 records directly (mixed-dtype
  tensor_tensor is exact on this HW).
- x ships as fp8 e4m3 (0.78%% of the metric per MB saved: the axon tunnel
  costs ~6.4ms/MB aggregate and dominates; fp8 x costs ~2e-3 rel err,
  budget is 2e-2).
- The folded-weight tail is SHARDED across the 8 cores (TS f32/core) and
  reassembled on device by a tiny AllGather that overlaps phase A; only a2w
  (128 f32, needed pre-AllGather) is replicated. Graph ids ship as u8.
- lidxs (scatter row ids) derived on device from lidxg instead of shipped.
- Softmax skips the segment-max subtraction (logits are bounded; pad slots
  carry a_s=-240), leaky-relu runs on the scalar engine as Prelu(alpha=0.2)
  (Lrelu ignores alpha on this HW), and for the single-head layer the a_d
  add is fused into Prelu's bias and the denominator into Exp's accum_out.
  One bulk [128, SK] DMA loads all gather ids. Work pools are triple-
  buffered; per-instruction latency (not throughput) is the device limit.
Shipped per core: ~0.55MB vs 0.73MB in v1 (~4.44MB total, ~6.4ms/MB).
Everything else (padded CSR per 128-node tile, degree-descending order,
per-k-slot indirect DMA gathers - multi-offset indirect DMA and
InstDMAGatherAnt are broken/unavailable on this runtime - on-the-fly
mean-pool matmul, AllReduce + replicated MLP head) matches v1.
"""
import numpy as np

import jax
jax.config.update("jax_compilation_cache_dir", "/tmp/jax_cache")
jax.config.update("jax_persistent_cache_min_compile_time_secs", 0.0)
jax.config.update("jax_persistent_cache_min_entry_size_bytes", 0)
from jax.experimental.shard_map import shard_map
from jax.sharding import Mesh, PartitionSpec

import concourse.bass as bass
import concourse.tile as ctile
from concourse import bass2jax, mybir


_NEFF_CACHE_DIR = "/tmp/bass_neff_cache"
_orig_compile_bir_kernel = bass2jax.compile_bir_kernel


def _cached_compile_bir_kernel(bir_json, tmpdir, neff_name="file.neff"):
    import hashlib as _hl, os as _os, shutil as _sh
    key = _hl.sha256(bir_json).hexdigest()
    cpath = _os.path.join(_NEFF_CACHE_DIR, f"{key}_{neff_name}")
    if _os.path.exists(cpath):
        out = _os.path.join(tmpdir, neff_name)
        _sh.copyfile(cpath, out)
        return out
    res = _orig_compile_bir_kernel(bir_json, tmpdir, neff_name)
    _os.makedirs(_NEFF_CACHE_DIR, exist_ok=True)
    tmp = f"{cpath}.tmp{_os.getpid()}"
    _sh.copyfile(res, tmp)
    _os.replace(tmp, cpath)
    return res


bass2jax.compile_bir_kernel = _cached_compile_bir_kernel
from concourse.vector_clock import ScopedClock
from concourse.masks import make_identity

F32 = mybir.dt.float32
I32 = mybir.dt.int32
U16 = mybir.dt.uint16
BF16 = mybir.dt.bfloat16
AX = mybir.AxisListType
OP = mybir.AluOpType
ACT = mybir.ActivationFunctionType

N = 50000
E0 = 1_600_000
G = 100
IN = 16
H1, C1 = 4, 32
F1 = H1 * C1              # 128
C2 = 32
NEG = 0.2
NC = 8
NL = N // NC              # 6250
P = 128
NT = (NL + P - 1) // P    # 49
NLP = NT * P              # 6272
REC1 = 20                 # bf16 [x(16), a_s1(4)]
REC2 = 36                 # bf16 [h2(32), a_s2(1), pad(3)]
TS = 1880                 # weight-tail f32 elements shipped per core


# ---------------------------------------------------------------------------
# TileContext tail-drain patch: this walrus build allows only one sem wait per
# CTRL instruction; spread the kernel-tail drain waits over several drains.
def _patched_drain_and_barrier(self, tick_clock, wait_clock):
    drain_inst = self.nc.sync.drain()
    extras = [self.nc.sync.drain() for _ in range(40)]
    wait_clock.add_sem_waits(
        drain_inst.ins, ScopedClock({None: tick_clock.global_clock})
    )
    si = drain_inst.ins.sync_info
    waits = list(si.on_wait or []) if si is not None else []
    if len(waits) > 1:
        si.on_wait = waits[:1]
        for i, w in enumerate(waits[1:]):
            esi = extras[i].ins.sync_info
            if esi is None:
                extras[i].ins.sync_info = mybir.SyncInfo(on_wait=[w], on_update=[])
            else:
                esi.on_wait = [w]
    self.nc.all_engine_barrier()
    popped = self.nc._tile_sem_poison_stack.pop()
    assert popped is self._sem_poison
    self.nc.clear_and_free_semaphores(list(self.sems.allocated().values()))
    self.nc.all_engine_barrier()


ctile.TileContext._drain_and_barrier = _patched_drain_and_barrier


def fix_multiwait(nc):
    """This walrus build allows only one sem wait per instruction: hoist all
    but one wait of any instruction onto same-engine NOPs inserted before it."""
    for f in nc.m.functions:
        for bb in f.blocks:
            lst = bb.instructions
            i = 0
            while i < len(lst):
                inst = lst[i]
                si = inst.sync_info
                waits = list(si.on_wait) if si and si.on_wait else []
                if len(waits) > 1:
                    si.on_wait = waits[-1:]
                    for w in waits[:-1]:
                        nop = mybir.InstNoOp(
                            name=nc.get_next_instruction_name(), ins=[], outs=[])
                        nop.engine = inst.engine
                        nop.sync_info = mybir.SyncInfo(on_wait=[w], on_update=[])
                        nc.register_instruction(nop)
                        lst.insert(i, nop)
                        i += 1
                i += 1


def vap(t, off, dims):
    """Flat (DRAM) AP view with extra element offset and [step,count] dims."""
    a = t[:] if not isinstance(t, bass.AP) else t
    return bass.AP(tensor=a.tensor, offset=a.offset + off, ap=dims)


def svap(t, off, free_dims):
    """SBUF AP view: keeps the base AP's partition pair (partition step must
    stay the tile's free pitch), custom free [step,count] dims + elem offset."""
    a = t[:] if not isinstance(t, bass.AP) else t
    return bass.AP(tensor=a.tensor, offset=a.offset + off,
                   ap=[list(a.ap[0])] + free_dims)


# ---------------------------------------------------------------------------
# input blob layouts (element offsets), shared by host packing and device code
def _layouts(Ks):
    L1TOT = P * int(sum(Ks))
    ib, off = {}, 0
    for name, sz in [("idx", L1TOT), ("lidxg", P * NT), ("xT", IN * NL // 2),
                     ("gidf", P * (NT + 1) // 2), ("tailsh", 2 * TS),
                     ("a2w", 2 * IN * 8)]:
        ib[name] = off
        off += sz
    return ib, off


# offsets of the folded weights inside the AllGather-reassembled f32 tail
WOFF = {}
_o = 0
for _n, _s in [("w1blk", 64 * F1), ("b1", F1), ("w2", F1 * C2),
               ("att2", C2 * 2), ("b2", C2), ("wh1", C2 * 64), ("bh1", 64),
               ("wh2", 64), ("bh2", 1), ("cnt", G), ("iota", G),
               ("piota", P)]:
    WOFF[_n] = _o
    _o += _s
assert _o <= NC * TS


def _to_bf16_bits(a):
    import ml_dtypes
    return np.asarray(a, np.float32).astype(ml_dtypes.bfloat16).view(np.uint16)


def _to_f8_bits(a):
    """fp8 e4m3 bits packed in pairs as u16 (little-endian byte order)."""
    import ml_dtypes
    b = np.asarray(a, np.float32).astype(ml_dtypes.float8_e4m3fn).view(np.uint8)
    return np.ascontiguousarray(b).view(np.uint16)


# ---------------------------------------------------------------------------
# host preprocessing: pure index/layout work, all vectorized
def host_prep(x, edge_index, batch):
    x = np.asarray(x, np.float32)
    ei = np.asarray(edge_index).astype(np.int64)
    batch = np.asarray(batch).astype(np.int64)

    src = np.concatenate([ei[0], np.arange(N, dtype=np.int64)]).astype(np.int32)
    dst = np.concatenate([ei[1], np.arange(N, dtype=np.int64)]).astype(np.int32)
    order = np.argsort(dst, kind="stable")
    src_s, dst_s = src[order], dst[order]
    deg = np.bincount(dst_s, minlength=N)
    rowptr = np.zeros(N + 1, np.int64)
    np.cumsum(deg, out=rowptr[1:])

    perms = np.empty((NC, NL), np.int32)
    invs = np.empty((NC, NL), np.int32)
    degs_sorted = np.empty((NC, NL), np.int64)
    for c in range(NC):
        d = deg[c * NL:(c + 1) * NL]
        p_ = np.argsort(-d, kind="stable").astype(np.int32)
        perms[c] = p_
        invs[c, p_] = np.arange(NL, dtype=np.int32)
        degs_sorted[c] = d[p_]

    heads = degs_sorted[:, ::P][:, :NT]                    # [NC, NT]
    Ks = np.maximum(4, ((heads.max(0) + 1) // 2) * 2).astype(np.int64)
    koff = np.zeros(NT + 1, np.int64)
    np.cumsum(Ks, out=koff[1:])
    SK = int(koff[-1])
    L1TOT = P * SK

    lv = np.arange(NLP)
    valid = lv < NL
    lv_c = np.minimum(lv, NL - 1)

    ibufs = []
    for c in range(NC):
        lo = c * NL
        e0, e1 = int(rowptr[lo]), int(rowptr[lo + NL])
        dloc = dst_s[e0:e1].astype(np.int64) - lo
        ks = np.arange(e0, e1, dtype=np.int64) - rowptr[lo + dloc]
        l = invs[c, dloc].astype(np.int64)
        # partition-major layout: one bulk [P, SK] DMA loads every tile's ids
        pos = (l & 127) * SK + koff[l >> 7] + ks
        idxf = np.full(L1TOT, N, np.uint16)
        idxf[pos] = src_s[e0:e1].astype(np.uint16)

        pv = perms[c][lv_c].astype(np.int64)
        lidxg = np.where(valid, pv, 0).reshape(NT, P).T.astype(np.uint16)
        # graph ids fit u8 (0..99; 255 = invalid row, matches no iota column);
        # pad NT=49 to 50 cols so u8 pairs pack into u16 words per row
        gidf = np.full((P, NT + 1), 255, np.uint8)
        gidf[:, :NT] = np.where(valid, batch[lo + pv], 255).reshape(NT, P).T
        gid8 = np.ascontiguousarray(gidf).view(np.uint16)

        ibufs.append(np.concatenate(
            [idxf, lidxg.ravel(),
             _to_f8_bits(np.ascontiguousarray(x[lo:lo + NL].T).ravel()),
             gid8.ravel()]))

    cnt = np.bincount(batch, minlength=G).astype(np.float32)
    return dict(Ks=[int(k) for k in Ks], ibufs=ibufs, cnt=cnt)


def fold_weights(W1, att_src1, att_dst1, b1, W2, att_src2, att_dst2, b2,
                 Wh1, bh1, Wh2, bh2, cnt):
    W1 = np.asarray(W1, np.float32)
    W1r = W1.reshape(IN, H1, C1)
    Vs = np.einsum("fhc,hc->fh", W1r, np.asarray(att_src1, np.float32))
    Vd = np.einsum("fhc,hc->fh", W1r, np.asarray(att_dst1, np.float32))
    a2w = np.concatenate([Vs, Vd], 1)                      # [16, 8]
    W1blk = np.zeros((64, F1), np.float32)
    for h in range(H1):
        W1blk[h * IN:(h + 1) * IN, h * C1:(h + 1) * C1] = W1r[:, h, :]
    att2 = np.stack([np.asarray(att_src2, np.float32).ravel(),
                     np.asarray(att_dst2, np.float32).ravel()], 1)  # [32, 2]
    tail = np.concatenate([
        W1blk.ravel(), np.asarray(b1, np.float32).ravel(),
        np.asarray(W2, np.float32).ravel(), att2.ravel(),
        np.asarray(b2, np.float32).ravel(),
        np.asarray(Wh1, np.float32).ravel(),
        np.asarray(bh1, np.float32).ravel(),
        np.asarray(Wh2, np.float32).ravel(),
        np.asarray(bh2, np.float32).ravel(),
        cnt.ravel(), np.arange(G, dtype=np.float32),
        np.arange(P, dtype=np.float32)])
    full = np.zeros(NC * TS, np.float32)
    full[:tail.size] = tail
    shards = full.reshape(NC, TS)
    a2w_bits = a2w.ravel().view(np.uint16)
    return [np.concatenate([shards[c].view(np.uint16), a2w_bits])
            for c in range(NC)]


# ---------------------------------------------------------------------------
def edge_softmax_aggregate(nc, pools, it_ap, tbl_dram, a_d_view, K, KM,
                           rec, nmsg, nheads, out_cb):
    """Per-tile padded-CSR gather + segment softmax + weighted aggregation.

    Gathers bf16 records, converts once to f32, then softmax+aggregate.
    No segment-max subtraction: logits are bounded (|a_s+a_d| <~ 10) so
    exp() is safe in f32 and alpha is shift-invariant; pad slots carry
    a_s = -240 -> exp(leaky(-240+a_d)) ~ 2e-21, negligible vs den >= e^-40.
    Leaky-relu runs on the scalar engine (Prelu, alpha=NEG; Lrelu ignores alpha on this HW); for nheads==1
    the a_d add is fused into Lrelu's bias and the denominator into Exp's
    accum_out, so the DVE only does reciprocal + prod + agg + scale.
    it_ap: [128, K] i32 SBUF AP of gather row ids.
    a_d_view: AP [128, nheads] f32 (per-dst attention term, this tile)
    rec: record width (bf16 elems); nmsg: message feature count;
    a_s lives at record col nmsg..nmsg+nheads-1.
    out_cb(OPS): callback receiving [128, nheads*nmsg] aggregated+normalized.
    """
    work = pools["work"]
    H = nheads
    g16 = work.tile([P, KM * rec], BF16, tag="g16")
    for k in range(K):
        nc.gpsimd.indirect_dma_start(
            out=g16[:, k * rec:(k + 1) * rec], out_offset=None, in_=tbl_dram,
            in_offset=bass.IndirectOffsetOnAxis(ap=it_ap[:, k:k + 1], axis=0))

    EX = work.tile([P, H * KM], F32, tag="EX")
    dr = work.tile([P, H], F32, tag="dr")
    if H == 1:
        Lm = work.tile([P, KM], F32, tag="Lm")
        nc.scalar.activation(Lm[:, :K], svap(g16, nmsg, [[rec, K]]),
                             ACT.Prelu, bias=a_d_view, alpha=NEG)
        den = work.tile([P, 1], F32, tag="den")
        nc.scalar.activation(EX[:, :K], Lm[:, :K], ACT.Exp, accum_out=den[:])
        nc.vector.reciprocal(dr[:], den[:])
    else:
        L0 = work.tile([P, H * KM], F32, tag="L0")
        nc.vector.tensor_tensor(
            out=L0[:, :H * K],
            in0=svap(g16, nmsg, [[1, H], [rec, K]]),
            in1=svap(a_d_view, 0, [[1, H], [0, K]]),
            op=OP.add)
        Lm = work.tile([P, H * KM], F32, tag="Lm")
        nc.scalar.activation(Lm[:, :H * K], L0[:, :H * K], ACT.Prelu, alpha=NEG)
        nc.scalar.activation(EX[:, :H * K], Lm[:, :H * K], ACT.Exp)
        den = work.tile([P, H], F32, tag="den")
        nc.vector.tensor_reduce(
            out=den[:], in_=svap(EX, 0, [[K, H], [1, K]]),
            axis=AX.X, op=OP.add)
        nc.vector.reciprocal(dr[:], den[:])
    # weighted aggregation: OP[p,h,f] = sum_k EX[p,h,k] * msg[p,k,f]
    prod = work.tile([P, H * KM * nmsg], BF16, tag="prod")
    nc.vector.tensor_tensor(
        out=prod[:, :H * K * nmsg],
        in0=svap(EX, 0, [[K, H], [1, K], [0, nmsg]]),
        in1=svap(g16, 0, [[0, H], [rec, K], [1, nmsg]]),
        op=OP.mult)
    agg = work.tile([P, H * nmsg], F32, tag="agg")
    nc.vector.tensor_reduce(
        out=agg[:],
        in_=svap(prod, 0, [[K * nmsg, H], [1, nmsg], [nmsg, K]]),
        axis=AX.X, op=OP.add)
    ops = work.tile([P, H * nmsg], F32, tag="ops")
    if H == 1:
        nc.vector.tensor_scalar_mul(ops[:], agg[:], dr[:])
    else:
        nc.vector.tensor_tensor(
            out=ops[:], in0=agg[:],
            in1=svap(dr, 0, [[1, H], [0, nmsg]]), op=OP.mult)
    out_cb(ops)


def build_fused(Ks):
    ib, LI = _layouts(Ks)
    SK = int(sum(Ks))
    koff = np.zeros(NT + 1, np.int64)
    np.cumsum(Ks, out=koff[1:])

    KM = int(max(Ks))
    nc = bass.Bass(num_devices=NC)
    # the axon tunnel moves several smaller parameters faster than one big
    # one: ship the blob as 8 equal chunks and restage into one DRAM scratch
    LI8 = ((LI + 8 * P - 1) // (8 * P)) * (8 * P)
    CHK = LI8 // 8
    ibufp = [nc.declare_dram_parameter(f"ibuf{i}", [CHK], U16, isOutput=False)
             for i in range(8)]
    out_d = nc.declare_dram_parameter("out", [1, G], F32, isOutput=True)
    ibuf = nc.dram_tensor("ibufd", [LI8], U16)

    wtsh = nc.dram_tensor("wtsh", [TS], F32)
    wtab = nc.dram_tensor("wtab", [NC * TS], F32, addr_space="Shared")

    T1slice = nc.dram_tensor("T1slice", [NL, REC1], BF16)
    T1 = nc.dram_tensor("T1", [N + 1, REC1], BF16, addr_space="Shared")
    adtab = nc.dram_tensor("adtab", [NL, 4], F32)
    T2slice = nc.dram_tensor("T2slice", [NL + 1, REC2], BF16)
    T2 = nc.dram_tensor("T2", [N + 1, REC2], BF16, addr_space="Shared")
    ad2d = nc.dram_tensor("ad2d", [NLP], F32)
    pool_in = nc.dram_tensor("pool_in", [G, C2], F32)
    pool_out = nc.dram_tensor("pool_out", [G, C2], F32, addr_space="Shared")

    GRP = [list(range(NC))]

    with ctile.TileContext(nc) as tc:
        import contextlib
        with contextlib.ExitStack() as ctx:
            const = ctx.enter_context(tc.tile_pool(name="const", bufs=1))
            persist = ctx.enter_context(tc.tile_pool(name="persist", bufs=1))
            work1 = ctx.enter_context(tc.tile_pool(name="work1", bufs=3))
            work2 = ctx.enter_context(tc.tile_pool(name="work2", bufs=3))
            psum = ctx.enter_context(tc.tile_pool(name="psum", bufs=4, space="PSUM"))
            ppool = ctx.enter_context(tc.tile_pool(name="ppool", bufs=1, space="PSUM"))
            pools1 = dict(work=work1, psum=psum)
            pools2 = dict(work=work2, psum=psum)

            ident = const.tile([P, P], F32)
            make_identity(nc, ident[:])

            # restage the 8 shipped chunks into the contiguous ibuf scratch
            for i in range(8):
                st = const.tile([P, CHK // P], U16, tag=f"stage{i}")
                nc.sync.dma_start(
                    out=st[:],
                    in_=vap(ibufp[i], 0, [[CHK // P, P], [1, CHK // P]]))
                nc.sync.dma_start(
                    out=vap(ibuf, i * CHK, [[CHK // P, P], [1, CHK // P]]),
                    in_=st[:])

            # ---- weight tail: sharded across cores; tiny AllGather that
            # overlaps phase A reassembles the full f32 tail on every core
            wt16 = const.tile([4, TS // 2], U16, tag="wt16")
            nc.sync.dma_start(
                out=wt16[:],
                in_=vap(ibuf, ib["tailsh"], [[TS // 2, 4], [1, TS // 2]]))
            nc.sync.dma_start(
                out=vap(wtsh, 0, [[TS // 4, 4], [1, TS // 4]]),
                in_=wt16[:].bitcast(F32))
            nc.gpsimd.collective_compute(
                "AllGather", OP.bypass, replica_groups=GRP,
                ins=[vap(wtsh, 0, [[1, TS]])],
                outs=[vap(wtab, 0, [[1, NC * TS]])])

            # a2w (phase-A matvec weights): replicated, straight from ibuf
            a2w16 = const.tile([IN, 16], U16, tag="a2w16")
            nc.sync.dma_start(
                out=a2w16[:], in_=vap(ibuf, ib["a2w"], [[16, IN], [1, 16]]))
            a2w_s = a2w16[:].bitcast(F32)

            def wload(name, shape, dims):
                t = const.tile(shape, F32, tag=f"w_{name}")
                nc.sync.dma_start(out=t[:], in_=vap(wtab, WOFF[name], dims))
                return t

            w1blk_s = wload("w1blk", [64, F1], [[F1, 64], [1, F1]])
            b1_s = wload("b1", [F1, 1], [[1, F1], [1, 1]])
            w2_s = wload("w2", [F1, C2], [[C2, F1], [1, C2]])
            att2_s = wload("att2", [C2, 2], [[2, C2], [1, 2]])
            b2bc_s = wload("b2", [P, C2], [[0, P], [1, C2]])
            wh1_s = wload("wh1", [C2, 64], [[64, C2], [1, 64]])
            bh1_s = wload("bh1", [64, 1], [[1, 64], [1, 1]])
            wh2_s = wload("wh2", [64, 1], [[1, 64], [1, 1]])
            bh2_s = wload("bh2", [1, 1], [[1, 1], [1, 1]])
            cnt_s = wload("cnt", [G, 1], [[1, G], [1, 1]])
            iota_s = wload("iota", [P, G], [[0, P], [1, G]])
            # NT=49 is odd: load u8-packed ids as NT+1 halfwords per row
            gid16 = const.tile([P, (NT + 1) // 2], U16)
            nc.sync.dma_start(
                out=gid16[:],
                in_=vap(ibuf, ib["gidf"], [[(NT + 1) // 2, P], [1, (NT + 1) // 2]]))
            gid_s = const.tile([P, NT + 1], F32)
            nc.vector.tensor_copy(out=gid_s[:], in_=gid16[:].bitcast(mybir.dt.uint8))
            lg16 = const.tile([P, NT], U16)
            nc.sync.dma_start(out=lg16[:], in_=vap(ibuf, ib["lidxg"], [[NT, P], [1, NT]]))
            lg = const.tile([P, NT], I32)
            nc.vector.tensor_copy(out=lg[:], in_=lg16[:])
            # scatter row ids = gather ids except the invalid tail rows
            # (local node id >= NL, i.e. p >= NL - (NT-1)*P in the last tile)
            # which scatter to the T2slice scratch row NL. Vector ops can't
            # start at partition 106, so blend with an is_lt(p, 106) mask.
            piota_s = wload("piota", [P, 1], [[1, P], [1, 1]])
            ls = const.tile([P, NT], I32)
            nc.vector.tensor_copy(out=ls[:], in_=lg[:])
            PV = NL - (NT - 1) * P
            vmask = const.tile([P, 1], F32, tag="vmask")
            nc.vector.tensor_scalar(
                out=vmask[:], in0=piota_s[:], scalar1=float(PV), scalar2=None,
                op0=OP.is_lt)
            lgf = const.tile([P, 1], F32, tag="lgf")
            nc.vector.tensor_copy(out=lgf[:], in_=lg[:, NT - 1:NT])
            lsf = const.tile([P, 1], F32, tag="lsf")
            nc.vector.scalar_tensor_tensor(
                out=lsf[:], in0=lgf[:], scalar=float(-NL), in1=vmask[:],
                op0=OP.add, op1=OP.mult)
            nc.vector.tensor_scalar(
                out=ls[:, NT - 1:NT], in0=lsf[:], scalar1=float(NL),
                scalar2=None, op0=OP.add)

            # ---- phase A: per-node logit terms for this core's nodes ----
            # x ships as fp8 e4m3 pairs (u16 words); NL and CH are even so
            # every chunk starts on a word boundary
            CH = 512
            for c0 in range(0, NL, CH):
                w = min(CH, NL - c0)
                x16c = work1.tile([IN, CH // 2], U16, tag="x16")
                nc.sync.dma_start(
                    out=x16c[:, :w // 2],
                    in_=vap(ibuf, ib["xT"] + c0 // 2, [[NL // 2, IN], [1, w // 2]]))
                xfc = work1.tile([IN, CH], F32, tag="xf")
                nc.vector.tensor_copy(
                    out=xfc[:, :w], in_=x16c[:, :w // 2].bitcast(mybir.dt.float8e4))
                pz = psum.tile([8, CH], F32, tag="ps")
                nc.tensor.matmul(pz[:, :w], lhsT=a2w_s, rhs=xfc[:, :w],
                                 start=True, stop=True)
                a8c = work1.tile([8, CH], F32, tag="a8c")
                nc.vector.tensor_copy(out=a8c[:, :w], in_=pz[:, :w])
                asb = work1.tile([4, CH], BF16, tag="asb")
                nc.vector.tensor_copy(out=asb[:, :w], in_=a8c[0:4, :w])
                xbc = work1.tile([IN, CH], BF16, tag="xb")
                nc.vector.tensor_copy(out=xbc[:, :w], in_=xfc[:, :w])
                nc.sync.dma_start(
                    out=vap(T1slice, c0 * REC1, [[1, IN], [REC1, w]]),
                    in_=xbc[:, :w])
                nc.sync.dma_start(
                    out=vap(T1slice, c0 * REC1 + IN, [[1, 4], [REC1, w]]),
                    in_=asb[:, :w])
                nc.sync.dma_start(
                    out=vap(adtab, c0 * 4, [[1, 4], [4, w]]), in_=a8c[4:8, :w])

            # per-dst a_d in degree-sorted order: [128, NT*4]
            adS = persist.tile([P, NT * 4], F32)
            for t in range(NT):
                nc.gpsimd.indirect_dma_start(
                    out=adS[:, t * 4:(t + 1) * 4], out_offset=None,
                    in_=adtab[:],
                    in_offset=bass.IndirectOffsetOnAxis(ap=lg[:, t:t + 1], axis=0))

            # ---- globalize T1 ----
            nc.gpsimd.collective_compute(
                "AllGather", OP.bypass, replica_groups=GRP,
                ins=[vap(T1slice, 0, [[1, NL * REC1]])],
                outs=[vap(T1, 0, [[1, N * REC1]])])
            # pad a_s = -240: exp(leaky(-240+a_d)) ~ 2e-21, no clamp needed
            dum1 = const.tile([1, REC1], BF16)
            nc.vector.memset(dum1[:], 0.0)
            nc.vector.memset(dum1[:, IN:IN + 4], -240.0)
            nc.sync.dma_start(
                out=vap(T1, N * REC1, [[REC1, 1], [1, REC1]]), in_=dum1[:])

            # ---- layer-1 edge phase ----
            it16all = persist.tile([P, SK], U16)
            nc.sync.dma_start(
                out=it16all[:], in_=vap(ibuf, ib["idx"], [[SK, P], [1, SK]]))
            idxall = persist.tile([P, SK], I32)
            nc.vector.tensor_copy(out=idxall[:], in_=it16all[:])
            h1e = persist.tile([F1, NLP], F32)
            for t in range(NT):
                K = Ks[t]
                o = int(koff[t])

                def finish1(ops, t=t):
                    pt = psum.tile([64, P], F32, tag="ps")
                    nc.tensor.transpose(out=pt[:], in_=ops[:], identity=ident[:, :P])
                    opst = work1.tile([64, P], F32, tag="opst")
                    nc.vector.tensor_copy(out=opst[:], in_=pt[:])
                    hz = psum.tile([F1, P], F32, tag="ps")
                    nc.tensor.matmul(hz[:], lhsT=w1blk_s[:], rhs=opst[:],
                                     start=True, stop=True)
                    zb = work1.tile([F1, P], F32, tag="zb")
                    nc.scalar.activation(zb[:], hz[:], ACT.Identity, bias=b1_s[:])
                    tmin = work1.tile([F1, P], F32, tag="tmin")
                    nc.vector.tensor_scalar_min(tmin[:], zb[:], 0.0)
                    te = work1.tile([F1, P], F32, tag="te")
                    nc.scalar.activation(te[:], tmin[:], ACT.Exp)
                    trelu = work1.tile([F1, P], F32, tag="trelu")
                    nc.vector.tensor_scalar_max(trelu[:], zb[:], 0.0)
                    nc.vector.scalar_tensor_tensor(
                        out=h1e[:, t * P:(t + 1) * P], in0=te[:], scalar=-1.0,
                        in1=trelu[:], op0=OP.add, op1=OP.add)

                edge_softmax_aggregate(
                    nc, pools1, idxall[:, o:o + K], T1[:],
                    adS[:, t * 4:(t + 1) * 4], K, KM, REC1, IN, H1, finish1)

            # ---- layer-2 node phase ----
            h2a = persist.tile([C2 + 1, NLP], F32)
            adrow = persist.tile([1, NLP], F32)
            for c0 in range(0, NLP, CH):
                w = min(CH, NLP - c0)
                pz = psum.tile([C2, CH], F32, tag="ps")
                nc.tensor.matmul(pz[:, :w], lhsT=w2_s[:], rhs=h1e[:, c0:c0 + w],
                                 start=True, stop=True)
                nc.vector.tensor_copy(out=h2a[0:C2, c0:c0 + w], in_=pz[:, :w])
                pa = psum.tile([1, CH], F32, tag="ps")
                nc.tensor.matmul(pa[:, :w], lhsT=att2_s[:, 0:1],
                                 rhs=h2a[0:C2, c0:c0 + w], start=True, stop=True)
                nc.vector.tensor_copy(out=h2a[C2:C2 + 1, c0:c0 + w], in_=pa[:, :w])
                pb = psum.tile([1, CH], F32, tag="ps")
                nc.tensor.matmul(pb[:, :w], lhsT=att2_s[:, 1:2],
                                 rhs=h2a[0:C2, c0:c0 + w], start=True, stop=True)
                nc.vector.tensor_copy(out=adrow[:, c0:c0 + w], in_=pb[:, :w])

            # ---- T2 record assembly: scatter rows to original local ids ----
            for t in range(NT):
                pt = psum.tile([P, C2 + 1], F32, tag="ps")
                nc.tensor.transpose(
                    out=pt[:], in_=h2a[:, t * P:(t + 1) * P],
                    identity=ident[0:C2 + 1, 0:C2 + 1])
                rec = work2.tile([P, REC2], BF16, tag="rec")
                nc.vector.tensor_copy(out=rec[:, 0:C2 + 1], in_=pt[:])
                nc.vector.memset(rec[:, C2 + 1:REC2], 0.0)
                nc.gpsimd.indirect_dma_start(
                    out=T2slice[:],
                    out_offset=bass.IndirectOffsetOnAxis(ap=ls[:, t:t + 1], axis=0),
                    in_=rec[:], in_offset=None)

            # per-dst a_d2 in degree-sorted order: bounce [1, NLP] -> [128, NT]
            nc.sync.dma_start(out=ad2d[:], in_=adrow[:])
            ad2S = persist.tile([P, NT], F32)
            nc.sync.dma_start(out=ad2S[:], in_=vap(ad2d, 0, [[1, P], [P, NT]]))

            # ---- globalize T2 ----
            nc.gpsimd.collective_compute(
                "AllGather", OP.bypass, replica_groups=GRP,
                ins=[vap(T2slice, 0, [[1, NL * REC2]])],
                outs=[vap(T2, 0, [[1, N * REC2]])])
            dum2 = const.tile([1, REC2], BF16)
            nc.vector.memset(dum2[:], 0.0)
            nc.vector.memset(dum2[:, C2:C2 + 1], -240.0)
            nc.sync.dma_start(
                out=vap(T2, N * REC2, [[REC2, 1], [1, REC2]]), in_=dum2[:])

            # ---- layer-2 edge phase + on-the-fly mean-pool matmul ----
            pooled = ppool.tile([G, C2], F32)
            for t in range(NT):
                K = Ks[t]
                o = int(koff[t])

                def finish2(ops, t=t):
                    zb = work2.tile([P, C2], F32, tag="zb2")
                    nc.vector.tensor_tensor(out=zb[:], in0=ops[:], in1=b2bc_s[:],
                                            op=OP.add)
                    tmin = work2.tile([P, C2], F32, tag="tmin2")
                    nc.vector.tensor_scalar_min(tmin[:], zb[:], 0.0)
                    te = work2.tile([P, C2], F32, tag="te2")
                    nc.scalar.activation(te[:], tmin[:], ACT.Exp)
                    trelu = work2.tile([P, C2], F32, tag="trelu2")
                    nc.vector.tensor_scalar_max(trelu[:], zb[:], 0.0)
                    hf = work2.tile([P, C2], F32, tag="hf")
                    nc.vector.scalar_tensor_tensor(
                        out=hf[:], in0=te[:], scalar=-1.0, in1=trelu[:],
                        op0=OP.add, op1=OP.add)
                    oh = work2.tile([P, G], F32, tag="oh")
                    nc.vector.tensor_tensor(
                        out=oh[:], in0=svap(gid_s, t, [[0, G]]),
                        in1=iota_s[:], op=OP.is_equal)
                    nc.tensor.matmul(
                        pooled[:], lhsT=oh[:], rhs=hf[:],
                        start=(t == 0), stop=(t == NT - 1))

                edge_softmax_aggregate(
                    nc, pools2, idxall[:, o:o + K], T2[:],
                    ad2S[:, t:t + 1], K, KM, REC2, C2, 1, finish2)

            # ---- AllReduce pooled partials + MLP head (redundant per-core) ----
            po = const.tile([G, C2], F32)
            nc.vector.tensor_copy(out=po[:], in_=pooled[:])
            nc.sync.dma_start(out=pool_in[:], in_=po[:])
            nc.gpsimd.collective_compute(
                "AllReduce", OP.add, replica_groups=GRP,
                ins=[vap(pool_in, 0, [[1, G * C2]])],
                outs=[vap(pool_out, 0, [[1, G * C2]])])
            sums = const.tile([G, C2], F32)
            nc.sync.dma_start(out=sums[:], in_=pool_out[:])
            cm = const.tile([G, 1], F32)
            nc.vector.tensor_scalar_max(cm[:], cnt_s[:], 1.0)
            nc.vector.reciprocal(cm[:], cm[:])
            pmean = const.tile([G, C2], F32)
            nc.vector.tensor_scalar_mul(pmean[:], sums[:], cm[:])

            pt = psum.tile([C2, G], F32, tag="ps")
            nc.tensor.transpose(out=pt[:], in_=pmean[:], identity=ident[:G, :G])
            pmeanT = const.tile([C2, G], F32)
            nc.vector.tensor_copy(out=pmeanT[:], in_=pt[:])
            z1 = psum.tile([64, G], F32, tag="ps")
            nc.tensor.matmul(z1[:], lhsT=wh1_s[:], rhs=pmeanT[:], start=True, stop=True)
            r1 = const.tile([64, G], F32)
            nc.scalar.activation(r1[:], z1[:], ACT.Relu, bias=bh1_s[:])
            z2 = psum.tile([1, G], F32, tag="ps")
            nc.tensor.matmul(z2[:], lhsT=wh2_s[:], rhs=r1[:], start=True, stop=True)
            o_ = const.tile([1, G], F32)
            nc.scalar.activation(o_[:], z2[:], ACT.Identity, bias=bh2_s[:])
            nc.sync.dma_start(out=out_d[:], in_=o_[:])
    fix_multiwait(nc)
    return nc


# ---------------------------------------------------------------------------
def _make_runner(nc, n_cores=NC):
    """Same execution path as bass2jax.run_bass_via_pjrt, but the jitted
    shard_map callable is built ONCE and reused."""
    bass2jax.install_neuronx_cc_hook()
    assert nc.dbg_addr is None
    partition_name = (nc.partition_id_tensor.name
                      if nc.partition_id_tensor else None)

    in_names, out_names, out_avals, zero_shapes = [], [], [], []
    for alloc in nc.m.functions[0].allocations:
        if not isinstance(alloc, mybir.MemoryLocationSet):
            continue
        name = alloc.memorylocations[0].name
        if alloc.kind == "ExternalInput":
            if name != partition_name:
                in_names.append(name)
        elif alloc.kind == "ExternalOutput":
            out_names.append(name)
            shape = tuple(alloc.tensor_shape)
            dtype = mybir.dt.np(alloc.dtype)
            out_avals.append(jax.core.ShapedArray(shape, dtype))
            zero_shapes.append((shape, dtype))
    n_params = len(in_names)
    n_outs = len(out_avals)
    all_names = in_names + out_names + ([partition_name] if partition_name else [])
    donate = tuple(range(n_params, n_params + n_outs))

    def _body(*args):
        operands = list(args)
        if partition_name is not None:
            operands.append(bass2jax.partition_id_tensor())
        outs = bass2jax._bass_exec_p.bind(
            *operands,
            out_avals=tuple(out_avals),
            in_names=tuple(all_names),
            out_names=tuple(out_names),
            lowering_input_output_aliases=(),
            sim_require_finite=True,
            sim_require_nnan=True,
            nc=nc,
        )
        return tuple(outs)

    devices = jax.devices()[:n_cores]
    mesh = Mesh(np.asarray(devices), ("core",))
    in_specs = (PartitionSpec("core"),) * (n_params + n_outs)
    out_specs = (PartitionSpec("core"),) * n_outs
    sharded = jax.jit(
        shard_map(_body, mesh=mesh, in_specs=in_specs, out_specs=out_specs,
                  check_rep=False),
        donate_argnums=donate, keep_unused=True)

    def run(in_maps):
        concat_in = [
            np.concatenate([np.asarray(m[name]) for m in in_maps], axis=0)
            for name in in_names]
        concat_zeros = [np.zeros((n_cores * s[0], *s[1:]), d)
                        for s, d in zero_shapes]
        out_arrs = sharded(*concat_in, *concat_zeros)
        return [
            {name: np.asarray(out_arrs[i]).reshape(n_cores, *out_avals[i].shape)[c]
             for i, name in enumerate(out_names)}
            for c in range(n_cores)]

    return run


_FUSED_CACHE = {}


def _get_fused(Ks):
    key = tuple(Ks)
    if key not in _FUSED_CACHE:
        nc = build_fused(list(Ks))
        _FUSED_CACHE[key] = (nc, _make_runner(nc))
    return _FUSED_CACHE[key]


def _make_inmaps(prep, tails):
    maps = []
    for c in range(NC):
        blob = np.concatenate([prep["ibufs"][c], tails[c]])
        li8 = ((blob.size + 8 * P - 1) // (8 * P)) * (8 * P)
        full = np.zeros(li8, np.uint16)
        full[:blob.size] = blob
        chk = li8 // 8
        maps.append({f"ibuf{i}": full[i * chk:(i + 1) * chk]
                     for i in range(8)})
    return maps


def kernel(x, edge_index, batch, W1, att_src1, att_dst1, b1,
           W2, att_src2, att_dst2, b2, Wh1, bh1, Wh2, bh2):
    prep = host_prep(x, edge_index, batch)
    tails = fold_weights(W1, att_src1, att_dst1, b1, W2, att_src2, att_dst2,
                         b2, Wh1, bh1, Wh2, bh2, prep["cnt"])
    _, run = _get_fused(prep["Ks"])
    in_maps = _make_inmaps(prep, tails)
    results = run(in_maps)
    return results[0]["out"].reshape(G, 1).astype(np.float32)


def _wall_min(fn, n=4):
    import time
    best = 1e9
    for _ in range(n):
        t0 = time.perf_counter()
        fn()
        best = min(best, time.perf_counter() - t0)
    return best


def _null_nc():
    nc = bass.Bass()
    x = nc.declare_dram_parameter("x", [P, 64], F32, isOutput=False)
    y = nc.declare_dram_parameter("y", [P, 64], F32, isOutput=True)
    with ctile.TileContext(nc) as tc:
        with tc.tile_pool(name="sbuf", bufs=1) as pool:
            t = pool.tile([P, 64], F32)
            nc.sync.dma_start(out=t[:], in_=x[:])
            nc.sync.dma_start(out=y[:], in_=t[:])
    fix_multiwait(nc)
    return nc


def timed_run(inputs):
    """Estimate on-device exec ns: warm per-call wall minus null-kernel wall.

    The axon PJRT path exposes no NTFF profiling, so this is an upper-bound
    estimate: warm wall of the single fused launch minus the warm wall of a
    trivial kernel (same dispatch/tunnel overhead), floored at 0.
    """
    prep = host_prep(inputs["x"], inputs["edge_index"], inputs["batch"])
    tails = fold_weights(inputs["W1"], inputs["att_src1"], inputs["att_dst1"],
                         inputs["b1"], inputs["W2"], inputs["att_src2"],
                         inputs["att_dst2"], inputs["b2"], inputs["Wh1"],
                         inputs["bh1"], inputs["Wh2"], inputs["bh2"],
                         prep["cnt"])
    in_maps = _make_inmaps(prep, tails)

    nc0 = _null_nc()
    run0 = _make_runner(nc0)
    im0 = [dict(x=np.zeros((P, 64), np.float32)) for _ in range(NC)]
    _, run1 = _get_fused(prep["Ks"])
    run0(im0)
    run1(in_maps)
    # interleave null/fused samples so slow drift in the axon tunnel rate
    # hits both measurements equally
    import time
    t0 = t1 = 1e9
    for _ in range(10):
        s = time.perf_counter()
        run0(im0)
        t0 = min(t0, time.perf_counter() - s)
        s = time.perf_counter()
        run1(in_maps)
        t1 = min(t1, time.perf_counter() - s)

    d1 = max(t1 - t0, 0.0)
    mb = sum(a.nbytes for m in in_maps for a in m.values()) / 1e6
    print(f"null wall {t0*1e3:.1f} ms; fused {t1*1e3:.1f} ms; "
          f"shipped {mb:.2f} MB")
    print(f"fused exec est {d1*1e6:.0f} us")
    return d1 * 1e9
